# revision 7
# baseline (speedup 1.0000x reference)
"""Trainium2 Bass kernel for ItemEmbeddingLayer (embedding_lookup).

Reference computation:
    out = Q_matrix[items] @ skill_embedding[user]      # [8192, 128] f32

Sharding: items split 1024/core across 8 cores (data parallel); Q (bf16,
exact - Q is binary) and the single user's embedding row (bf16) replicated.

Per-core device kernel (computes out' = out^T; host transposes back):
  1. Load the FULL Q, host-transposed to Qt[p, r, c] = Q[r, c*128+p]
     ([128, 4096, 2] bf16 = 2MB), as one plain 128-descriptor DMA. This
     avoids per-row gather descriptors entirely: SWDGE descriptor
     generation on the Pool engine costs ~9ns/row however it is batched
     (8 x indirect_dma_start = 8.8us, 2 x dma_gather = 9.6us measured),
     while a plain 2MB DMA is pure bandwidth (~5.8us) and needs no
     index data, so it starts at tile-context open and overlaps the
     gpsimd library load + idx/emb loads.
  2. TWO ap_gather ops (Pool engine, free-dim SBUF gather, 8 Q7 cpus in
     parallel, idxs replicated per 16-partition window):
     qg_r[p, i, c] = Qt[p, items[r*512+i], c].
  3. Matmuls with the embedding as stationary weights and stride-2
     moving APs: ps[k, i] += emb[c*128+p, k]^T . qg_r[p, i, c], both
     skill-chunks accumulated in fp32 PSUM, two 512-wide output regions.
  4. DVE copies PSUM->SBUF as bf16, 2D DMA out per region (sync/scalar).
Host: concat per-core [128, 1024] -> [128, 8192] -> transpose -> [8192, 128].
"""

import numpy as np
import ml_dtypes

import concourse.bass as bass
import concourse.bacc as bacc
import concourse.mybir as mybir
from concourse.tile import TileContext
from concourse.bass_utils import run_bass_kernel_spmd

N_CORES = 8
L = 8192
LC = L // N_CORES          # 1024 items per core
S = 256
K = 128
R = 4096
P = 128
IW = LC // 16              # idx tile free dim (wrapped in 16 partitions)

# output regions (start_col, end_col); each must fit one PSUM bank (<=512 f32)
REGIONS = [(0, 512), (512, 1024)]


def build_bass() -> bass.Bass:
    nc = bacc.Bacc(trn_type="TRN2", dynamic_dma_scratch_size=131072)
    qt = nc.declare_dram_parameter("qt", [P, R, 2], mybir.dt.bfloat16, isOutput=False)
    idx = nc.declare_dram_parameter("idx", [P, IW], mybir.dt.int16, isOutput=False)
    emb = nc.declare_dram_parameter("emb", [P, 2, K], mybir.dt.bfloat16, isOutput=False)
    out = nc.declare_dram_parameter("out", [K, LC], mybir.dt.bfloat16, isOutput=True)

    with (
        TileContext(nc) as tc,
        tc.tile_pool(name="main", bufs=1) as pool,
        tc.tile_pool(name="acc", bufs=1, space="PSUM") as apsum,
    ):
        qt_t = pool.tile([P, R, 2], mybir.dt.bfloat16)
        nc.scalar.dma_start(out=qt_t[:], in_=qt[:])
        idx_t = pool.tile([P, IW], mybir.dt.int16)
        nc.sync.dma_start(out=idx_t[:], in_=idx[:])
        emb_t = pool.tile([P, 2, K], mybir.dt.bfloat16)
        nc.sync.dma_start(out=emb_t[:], in_=emb[:])

        engs = [nc.sync, nc.scalar]
        for r, (c0, c1) in enumerate(REGIONS):
            n = c1 - c0
            qg = pool.tile([P, n, 2], mybir.dt.bfloat16, tag=f"qg{r}")
            nc.gpsimd.ap_gather(
                qg[:],
                qt_t[:],
                idx_t[:, c0 // 16 : c1 // 16],
                channels=P,
                num_elems=R,
                d=2,
                num_idxs=n,
            )
            ps = apsum.tile([P, n], mybir.dt.float32, tag=f"ps{r}")
            nc.tensor.matmul(ps[:], emb_t[:, 0, :], qg[:, :, 0], start=True, stop=False)
            nc.tensor.matmul(ps[:], emb_t[:, 1, :], qg[:, :, 1], start=False, stop=True)
            o = pool.tile([P, n], mybir.dt.bfloat16, tag=f"o{r}")
            nc.vector.tensor_copy(o[:], ps[:])
            engs[r % len(engs)].dma_start(out=out[:, c0:c1], in_=o[:])

    nc.compile()
    return nc


_CACHE: dict = {}


def get_nc() -> bass.Bass:
    if "nc" not in _CACHE:
        _CACHE["nc"] = build_bass()
    return _CACHE["nc"]


def make_in_maps(user, Q_matrix, items, skill_embedding):
    user = int(np.asarray(user))
    Q = np.asarray(Q_matrix, dtype=np.float32)
    items = np.asarray(items).astype(np.int64)
    E = np.ascontiguousarray(np.asarray(skill_embedding)[user], dtype=np.float32)

    # Qt[p, r, c] = Q[r, c*128+p]
    qt = np.ascontiguousarray(
        Q.reshape(R, 2, P).transpose(2, 0, 1).astype(ml_dtypes.bfloat16)
    )

    hi = E.astype(ml_dtypes.bfloat16)
    emb = np.empty((P, 2, K), dtype=ml_dtypes.bfloat16)
    for c in range(2):
        emb[:, c, :] = hi[c * P : (c + 1) * P, :]

    in_maps = []
    for i in range(N_CORES):
        it = items[i * LC : (i + 1) * LC].astype(np.int16)
        # [16, IW] block (element i at [i%16, i//16]), replicated across all
        # 8 Q7-cpu partition windows (each core reads idxs from its own window)
        idx_arr = np.ascontiguousarray(np.tile(it.reshape(IW, 16).T, (8, 1)))
        in_maps.append({"qt": qt, "idx": idx_arr, "emb": emb})
    return in_maps


def kernel(user, Q_matrix, items, skill_embedding, _trace=False, _result_box=None):
    in_maps = make_in_maps(user, Q_matrix, items, skill_embedding)
    res = run_bass_kernel_spmd(get_nc(), in_maps, list(range(N_CORES)), trace=_trace)
    if _result_box is not None:
        _result_box.append(res)
    full = np.concatenate(
        [np.asarray(res.results[i]["out"]).astype(np.float32) for i in range(N_CORES)],
        axis=1,
    )
    return np.ascontiguousarray(full.T, dtype=np.float32)


# revision 13
# speedup vs baseline: 1.6530x; 1.6530x over previous
"""Trainium2 Bass kernel for ItemEmbeddingLayer (embedding_lookup).

Reference computation:
    out = Q_matrix[items] @ skill_embedding[user]      # [8192, 128] f32

Sharding: items split 1024/core across 8 cores (data parallel); Q (bf16,
exact - Q is binary) and the single user's embedding row (bf16) replicated.

Per-core device kernel (computes out' = out^T; host transposes back):
  1. 8x indirect_dma_start gathers (128 rows each, the SWDGE per-instruction
     fixed cost makes this the pacing chain) -> q_sb[j] [128(item), 256(skill)]
  2. PE transposes each [128,128] block into [skill, item] layout (qT),
     DVE copies PSUM->SBUF, pipelined per chunk behind the gathers.
  3. Matmuls with the embedding as stationary weights:
     ps[k, l] += emb[s,c,k]^T . qT[s,c,l], both skill-chunks accumulated
     in fp32 PSUM. Output regions split 512/256/128/128 wide so the last
     gathered chunk has minimal dependent work (short tail).
  4. DVE copies PSUM->SBUF as bf16, 2D DMA out per region (sync/scalar).
Host: concat per-core [128, 1024] -> [128, 8192] -> transpose -> [8192, 128].
"""

import numpy as np
import ml_dtypes

import concourse.bass as bass
import concourse.bacc as bacc
import concourse.mybir as mybir
from concourse.tile import TileContext
from concourse.bass_utils import run_bass_kernel_spmd

N_CORES = 8
L = 8192
LC = L // N_CORES
S = 256
K = 128
R = 4096
P = 128
NCH = LC // P

PSUM_DMA = False    # DMA outputs straight from PSUM (no SBUF copy)
HILO = False       # single bf16 E (False) vs hi+lo split (True)
LAST_PSUM_F32 = False  # (dead end: DMA cannot read PSUM on this target)

# (start_chunk, end_chunk) per output region; region width = 128*(e-s)
REGIONS = [(0, 4), (4, 7), (7, 8)]


def build_bass() -> bass.Bass:
    nc = bacc.Bacc(trn_type="TRN2", dynamic_dma_scratch_size=131072)
    q = nc.declare_dram_parameter("q_bf16", [R, S], mybir.dt.bfloat16, isOutput=False)
    idx = nc.declare_dram_parameter("idx", [P, NCH], mybir.dt.int32, isOutput=False)
    nE = 2 if HILO else 1
    emb = nc.declare_dram_parameter("emb", [P, 2, nE * K], mybir.dt.bfloat16, isOutput=False)
    ident = nc.declare_dram_parameter("ident", [P, P], mybir.dt.bfloat16, isOutput=False)
    out = nc.declare_dram_parameter("out", [K, LC], mybir.dt.bfloat16, isOutput=True)
    last_n = (REGIONS[-1][1] - REGIONS[-1][0]) * P
    out2 = (
        nc.declare_dram_parameter("out2", [K, last_n], mybir.dt.float32, isOutput=True)
        if LAST_PSUM_F32
        else None
    )

    with (
        TileContext(nc) as tc,
        tc.tile_pool(name="main", bufs=1) as pool,
        tc.tile_pool(name="gat", bufs=NCH) as gpool,
        tc.tile_pool(name="tps", bufs=4, space="PSUM") as tpsum,
        tc.tile_pool(name="acc", bufs=1, space="PSUM") as apsum,
    ):
        idx_t = pool.tile([P, NCH], mybir.dt.int32)
        nc.sync.dma_start(out=idx_t[:], in_=idx[:])
        emb_t = pool.tile([P, 2, nE * K], mybir.dt.bfloat16)
        nc.scalar.dma_start(out=emb_t[:], in_=emb[:])
        ident_t = pool.tile([P, P], mybir.dt.bfloat16)
        nc.scalar.dma_start(out=ident_t[:], in_=ident[:])

        qT = pool.tile([P, 2, LC], mybir.dt.bfloat16)

        def mm_region(r):
            s0, s1 = REGIONS[r]
            n = (s1 - s0) * P
            ps = apsum.tile([P, n], mybir.dt.float32, tag=f"ps{r}")
            first = True
            for c in range(2):
                for e in range(nE):
                    nc.tensor.matmul(
                        ps[:],
                        emb_t[:, c, e * K : (e + 1) * K],
                        qT[:, c, s0 * P : s1 * P],
                        start=first,
                        stop=(c == 1 and e == nE - 1),
                    )
                    first = False
            engs = [nc.scalar, nc.scalar, nc.sync]
            last = r == len(REGIONS) - 1
            if LAST_PSUM_F32 and last:
                engs[r].dma_start(out=out2[:], in_=ps[:])
            elif PSUM_DMA:
                engs[r].dma_start(out=out[:, s0 * P : s1 * P], in_=ps[:])
            else:
                o = pool.tile([P, n], mybir.dt.bfloat16, tag=f"o{r}")
                nc.vector.tensor_copy(o[:], ps[:])
                engs[r].dma_start(out=out[:, s0 * P : s1 * P], in_=o[:])

        region_of_chunk = {}
        for r, (s0, s1) in enumerate(REGIONS):
            region_of_chunk[s1 - 1] = r

        for j in range(NCH):
            t = gpool.tile([P, S], mybir.dt.bfloat16, tag=f"q{j}")
            nc.gpsimd.indirect_dma_start(
                out=t[:],
                out_offset=None,
                in_=q[:],
                in_offset=bass.IndirectOffsetOnAxis(ap=idx_t[:, j : j + 1], axis=0),
            )
            for c in range(2):
                tp = tpsum.tile([P, P], mybir.dt.bfloat16, tag="tp")
                nc.tensor.transpose(
                    out=tp[:], in_=t[:, c * P : (c + 1) * P], identity=ident_t[:]
                )
                nc.vector.tensor_copy(qT[:, c, j * P : (j + 1) * P], tp[:])
            if j in region_of_chunk:
                mm_region(region_of_chunk[j])

    nc.compile()
    return nc


_CACHE: dict = {}


def get_nc() -> bass.Bass:
    if "nc" not in _CACHE:
        _CACHE["nc"] = build_bass()
    return _CACHE["nc"]


def make_in_maps(user, Q_matrix, items, skill_embedding):
    user = int(np.asarray(user))
    Q = np.asarray(Q_matrix, dtype=np.float32)
    items = np.asarray(items).astype(np.int64)
    E = np.ascontiguousarray(np.asarray(skill_embedding)[user], dtype=np.float32)
    q_bf = Q.astype(ml_dtypes.bfloat16)
    ident = np.eye(P, dtype=ml_dtypes.bfloat16)

    nE = 2 if HILO else 1
    hi = E.astype(ml_dtypes.bfloat16)
    emb = np.empty((P, 2, nE * K), dtype=ml_dtypes.bfloat16)
    for c in range(2):
        emb[:, c, 0:K] = hi[c * P : (c + 1) * P, :]
        if HILO:
            lo = (E - hi.astype(np.float32)).astype(ml_dtypes.bfloat16)
            emb[:, c, K : 2 * K] = lo[c * P : (c + 1) * P, :]

    in_maps = []
    for i in range(N_CORES):
        it = items[i * LC : (i + 1) * LC].astype(np.int32)
        idx_arr = np.ascontiguousarray(it.reshape(NCH, P).T)
        in_maps.append({"q_bf16": q_bf, "idx": idx_arr, "emb": emb, "ident": ident})
    return in_maps


def kernel(user, Q_matrix, items, skill_embedding, _trace=False, _result_box=None):
    in_maps = make_in_maps(user, Q_matrix, items, skill_embedding)
    res = run_bass_kernel_spmd(get_nc(), in_maps, list(range(N_CORES)), trace=_trace)
    if _result_box is not None:
        _result_box.append(res)
    last_n = (REGIONS[-1][1] - REGIONS[-1][0]) * P

    def core_out(i):
        o = np.asarray(res.results[i]["out"]).astype(np.float32)
        if LAST_PSUM_F32:
            o = np.concatenate(
                [o[:, : LC - last_n], np.asarray(res.results[i]["out2"])], axis=1
            )
        return o

    full = np.concatenate([core_out(i) for i in range(N_CORES)], axis=1)
    return np.ascontiguousarray(full.T, dtype=np.float32)



# revision 15
# speedup vs baseline: 1.7007x; 1.0289x over previous
"""Trainium2 Bass kernel for ItemEmbeddingLayer (embedding_lookup).

Reference computation:
    out = Q_matrix[items] @ skill_embedding[user]      # [8192, 128] f32

Sharding: items split 1024/core across 8 cores (data parallel); Q (bf16,
exact - Q is binary) and the single user's embedding row (bf16) replicated.

Per-core device kernel (computes out' = out^T; host transposes back):
  1. 8x indirect_dma_start gathers (128 rows each, the SWDGE per-instruction
     fixed cost makes this the pacing chain) -> q_sb[j] [128(item), 256(skill)]
  2. PE transposes each [128,128] block into [skill, item] layout (qT),
     DVE copies PSUM->SBUF, pipelined per chunk behind the gathers.
  3. Matmuls with the embedding as stationary weights:
     ps[k, l] += emb[s,c,k]^T . qT[s,c,l], both skill-chunks accumulated
     in fp32 PSUM. Output regions split 512/256/128/128 wide so the last
     gathered chunk has minimal dependent work (short tail).
  4. DVE copies PSUM->SBUF as bf16, 2D DMA out per region (sync/scalar).
Host: concat per-core [128, 1024] -> [128, 8192] -> transpose -> [8192, 128].
"""

import numpy as np
import ml_dtypes

import concourse.bass as bass
import concourse.bacc as bacc
import concourse.mybir as mybir
from concourse.tile import TileContext
from concourse.bass_utils import run_bass_kernel_spmd

N_CORES = 8
L = 8192
LC = L // N_CORES
S = 256
K = 128
R = 4096
P = 128
NCH = LC // P

PSUM_DMA = False    # DMA outputs straight from PSUM (no SBUF copy)
HILO = False       # single bf16 E (False) vs hi+lo split (True)
LAST_PSUM_F32 = False  # (dead end: DMA cannot read PSUM on this target)

# (start_chunk, end_chunk) per output region; region width = 128*(e-s)
REGIONS = [(0, 4), (4, 7), (7, 8)]


def build_bass() -> bass.Bass:
    nc = bacc.Bacc(trn_type="TRN2", dynamic_dma_scratch_size=131072)
    q = nc.declare_dram_parameter("q_bf16", [R, S], mybir.dt.bfloat16, isOutput=False)
    idx = nc.declare_dram_parameter("idx", [P, NCH], mybir.dt.int32, isOutput=False)
    nE = 2 if HILO else 1
    emb = nc.declare_dram_parameter("emb", [P, 2, nE * K], mybir.dt.bfloat16, isOutput=False)
    ident = nc.declare_dram_parameter("ident", [P, P], mybir.dt.bfloat16, isOutput=False)
    out = nc.declare_dram_parameter("out", [K, LC], mybir.dt.bfloat16, isOutput=True)
    last_n = (REGIONS[-1][1] - REGIONS[-1][0]) * P
    out2 = (
        nc.declare_dram_parameter("out2", [K, last_n], mybir.dt.float32, isOutput=True)
        if LAST_PSUM_F32
        else None
    )

    with (
        TileContext(nc) as tc,
        tc.tile_pool(name="main", bufs=1) as pool,
        tc.tile_pool(name="gat", bufs=NCH) as gpool,
        tc.tile_pool(name="tps", bufs=4, space="PSUM") as tpsum,
        tc.tile_pool(name="acc", bufs=1, space="PSUM") as apsum,
    ):
        idx_t = pool.tile([P, NCH], mybir.dt.int32)
        # idx via gpsimd's own SWDGE queue: the gathers (same engine) see its
        # completion without a cross-engine semaphore hop, starting the
        # indirect chain ~0.8us earlier than a sync-issued load.
        nc.gpsimd.dma_start(out=idx_t[:], in_=idx[:])
        emb_t = pool.tile([P, 2, nE * K], mybir.dt.bfloat16)
        nc.scalar.dma_start(out=emb_t[:], in_=emb[:])
        ident_t = pool.tile([P, P], mybir.dt.bfloat16)
        nc.scalar.dma_start(out=ident_t[:], in_=ident[:])

        qT = pool.tile([P, 2, LC], mybir.dt.bfloat16)

        def mm_region(r):
            s0, s1 = REGIONS[r]
            n = (s1 - s0) * P
            ps = apsum.tile([P, n], mybir.dt.float32, tag=f"ps{r}")
            first = True
            for c in range(2):
                for e in range(nE):
                    nc.tensor.matmul(
                        ps[:],
                        emb_t[:, c, e * K : (e + 1) * K],
                        qT[:, c, s0 * P : s1 * P],
                        start=first,
                        stop=(c == 1 and e == nE - 1),
                    )
                    first = False
            engs = [nc.sync, nc.sync, nc.scalar]
            last = r == len(REGIONS) - 1
            if LAST_PSUM_F32 and last:
                engs[r].dma_start(out=out2[:], in_=ps[:])
            elif PSUM_DMA:
                engs[r].dma_start(out=out[:, s0 * P : s1 * P], in_=ps[:])
            else:
                o = pool.tile([P, n], mybir.dt.bfloat16, tag=f"o{r}")
                nc.vector.tensor_copy(o[:], ps[:])
                engs[r].dma_start(out=out[:, s0 * P : s1 * P], in_=o[:])

        region_of_chunk = {}
        for r, (s0, s1) in enumerate(REGIONS):
            region_of_chunk[s1 - 1] = r

        for j in range(NCH):
            t = gpool.tile([P, S], mybir.dt.bfloat16, tag=f"q{j}")
            nc.gpsimd.indirect_dma_start(
                out=t[:],
                out_offset=None,
                in_=q[:],
                in_offset=bass.IndirectOffsetOnAxis(ap=idx_t[:, j : j + 1], axis=0),
            )
            for c in range(2):
                tp = tpsum.tile([P, P], mybir.dt.bfloat16, tag="tp")
                nc.tensor.transpose(
                    out=tp[:], in_=t[:, c * P : (c + 1) * P], identity=ident_t[:]
                )
                nc.vector.tensor_copy(qT[:, c, j * P : (j + 1) * P], tp[:])
            if j in region_of_chunk:
                mm_region(region_of_chunk[j])

    nc.compile()
    return nc


_CACHE: dict = {}


def get_nc() -> bass.Bass:
    if "nc" not in _CACHE:
        _CACHE["nc"] = build_bass()
    return _CACHE["nc"]


def make_in_maps(user, Q_matrix, items, skill_embedding):
    user = int(np.asarray(user))
    Q = np.asarray(Q_matrix, dtype=np.float32)
    items = np.asarray(items).astype(np.int64)
    E = np.ascontiguousarray(np.asarray(skill_embedding)[user], dtype=np.float32)
    q_bf = Q.astype(ml_dtypes.bfloat16)
    ident = np.eye(P, dtype=ml_dtypes.bfloat16)

    nE = 2 if HILO else 1
    hi = E.astype(ml_dtypes.bfloat16)
    emb = np.empty((P, 2, nE * K), dtype=ml_dtypes.bfloat16)
    for c in range(2):
        emb[:, c, 0:K] = hi[c * P : (c + 1) * P, :]
        if HILO:
            lo = (E - hi.astype(np.float32)).astype(ml_dtypes.bfloat16)
            emb[:, c, K : 2 * K] = lo[c * P : (c + 1) * P, :]

    in_maps = []
    for i in range(N_CORES):
        it = items[i * LC : (i + 1) * LC].astype(np.int32)
        idx_arr = np.ascontiguousarray(it.reshape(NCH, P).T)
        in_maps.append({"q_bf16": q_bf, "idx": idx_arr, "emb": emb, "ident": ident})
    return in_maps


def kernel(user, Q_matrix, items, skill_embedding, _trace=False, _result_box=None):
    in_maps = make_in_maps(user, Q_matrix, items, skill_embedding)
    res = run_bass_kernel_spmd(get_nc(), in_maps, list(range(N_CORES)), trace=_trace)
    if _result_box is not None:
        _result_box.append(res)
    last_n = (REGIONS[-1][1] - REGIONS[-1][0]) * P

    def core_out(i):
        o = np.asarray(res.results[i]["out"]).astype(np.float32)
        if LAST_PSUM_F32:
            o = np.concatenate(
                [o[:, : LC - last_n], np.asarray(res.results[i]["out2"])], axis=1
            )
        return o

    full = np.concatenate([core_out(i) for i in range(N_CORES)], axis=1)
    return np.ascontiguousarray(full.T, dtype=np.float32)

